# revision 1
# baseline (speedup 1.0000x reference)
"""GCN (3-layer DGL GraphConv, norm='both') on 8 TRN2 NeuronCores.

Strategy: nodes partitioned across cores by dst range. Per layer:
  - per-edge messages gathered via indirect DMA from a replicated node table
  - segment-sum over dst implemented as a triangular-matmul cumsum along
    partitions + two boundary gathers + subtract (no scatter needed; edges
    are host-packed so no node's run crosses a 128-slot column)
  - tiny dense GEMMs (W0 outer-product / W1 / W2) on PE
  - AllGather replicates each core's message-table slice between layers.
Host does index-only preprocessing (sort/pack/degree counts); all float
math runs on device.
"""

import sys

import numpy as np

try:
    import concourse.bass as bass  # noqa: F401
except Exception:  # pragma: no cover
    sys.path.insert(0, "/opt/trn_rl_repo")

import concourse.bass as bass
import concourse.bacc as bacc
import concourse.tile as tile
from concourse import mybir
from concourse.bass_utils import run_bass_kernel_spmd
from concourse.masks import make_upper_triangular

N_CORES = 8
N_NODES = 50000
NPC = N_NODES // N_CORES  # 6250 nodes per core
NPAD = 6272               # 49 * 128
NI = NPAD // 128          # 49
TBL = N_CORES * NPAD      # 50176 table rows
TW = TBL // 128           # 392
F1 = 100
F2 = 10
P = 128
GHOST_G = 6250                       # core-0 pad row, G layout (zeroed)
GHOST_K = (6250 % 128) * NI + 6250 // 128  # 5242, core-0 pad row, K layout
DT = mybir.dt.float32
IT = mybir.dt.int32


def _kappa(m):
    return (m % 128) * NI + m // 128


def _preprocess(edge_index):
    """Index-only host prep. Returns per-core dict arrays + W."""
    src = edge_index[0].astype(np.int64)
    dst = edge_index[1].astype(np.int64)
    deg_src = np.bincount(src, minlength=N_NODES).astype(np.float32)
    deg_dst = np.bincount(dst, minlength=N_NODES).astype(np.float32)
    order = np.argsort(dst, kind="stable")
    src_s = src[order]
    dst_s = dst[order]
    bounds = np.searchsorted(dst_s, np.arange(0, N_NODES + 1, NPC))

    packs = []
    Wmax = 0
    for c in range(N_CORES):
        lo, hi = bounds[c], bounds[c + 1]
        s_c = src_s[lo:hi]
        d_c = (dst_s[lo:hi] - c * NPC).astype(np.int64)
        lens = np.bincount(d_c, minlength=NPC).astype(np.int64)
        assert lens.max() <= 128, f"node degree {lens.max()} exceeds 128"
        # class-grouped FFD: class r = n%NI -> psum column r; rows q=n//NI
        # are distinct within a class, so a [slots, q] interval-mask matmul
        # per column accumulates segment sums directly in G-layout
        col = np.zeros(NPC, np.int64)
        slot0 = np.zeros(NPC, np.int64)
        ccols = np.zeros(NI, np.int64)
        for r in range(NI):
            nodes = np.arange(r, NPC, NI)
            fill = []
            for n in nodes[np.argsort(-lens[nodes], kind="stable")]:
                L = int(lens[n])
                if L == 0:
                    break
                for i in range(len(fill)):
                    if fill[i] + L <= 128:
                        col[n] = i
                        slot0[n] = fill[i]
                        fill[i] += L
                        break
                else:
                    col[n] = len(fill)
                    slot0[n] = 0
                    fill.append(L)
            ccols[r] = len(fill)
        packs.append((s_c, d_c, lens, col, slot0, ccols))

    CC = np.max([p[5] for p in packs], axis=0)  # shared per-class budgets
    OFF = np.zeros(NI + 1, np.int64)
    np.cumsum(CC, out=OFF[1:])
    W = int(OFF[-1])
    ZERO_POS = 128 * W
    cores = []
    for c in range(N_CORES):
        s_c, d_c, lens, col, slot0, _cc = packs[c]
        col = col + OFF[np.arange(NPC) % NI]  # class-local -> global column
        run_start = np.zeros(NPC + 1, np.int64)
        np.cumsum(lens, out=run_start[1:])
        e_n = d_c
        j_e = np.arange(len(s_c)) - run_start[e_n]
        p_e = slot0[e_n] + j_e
        i_e = col[e_n]
        sc_core = s_c // NPC
        sc_loc = s_c % NPC
        sidx_g = np.full((128, W), GHOST_G, np.int32)
        sidx_k = np.full((128, W), GHOST_K, np.int32)
        sidx_g[p_e, i_e] = (sc_core * NPAD + sc_loc).astype(np.int32)
        sidx_k[p_e, i_e] = (sc_core * NPAD + _kappa(sc_loc)).astype(np.int32)

        mask = np.zeros((128, W * 128), np.float32)
        for n in range(NPC):
            L = int(lens[n])
            if L:
                mask[slot0[n]:slot0[n] + L, col[n] * 128 + n // NI] = 1.0

        bnd_end = np.full(NPAD, ZERO_POS, np.int64)
        bnd_beg = np.full(NPAD, ZERO_POS, np.int64)
        has = lens > 0
        idxs = np.nonzero(has)[0]
        bnd_end[idxs] = (slot0[idxs] + lens[idxs] - 1) * W + col[idxs]
        bnd_beg[idxs] = np.where(
            slot0[idxs] > 0, (slot0[idxs] - 1) * W + col[idxs], ZERO_POS
        )

        dd_own = np.ones(NPAD, np.float32)
        dd_own[:NPC] = deg_dst[c * NPC : (c + 1) * NPC]
        # pad rows get a huge degree so inv_sqrt ~ 0 zeroes their messages
        # (they are the target of ghost-edge gathers in the next layer)
        dsg = np.full(NPAD, 1e30, np.float32)
        dsg[:NPC] = deg_src[c * NPC : (c + 1) * NPC]
        dsk = np.full(NPAD, 1e30, np.float32)
        dsk[_kappa(np.arange(NPC))] = deg_src[c * NPC : (c + 1) * NPC]
        cores.append(
            dict(
                sidx_g=sidx_g,
                sidx_k=sidx_k,
                mask=mask,
                bnd_end=bnd_end.astype(np.int32).reshape(128, NI),
                bnd_beg=bnd_beg.astype(np.int32).reshape(128, NI),
                deg_dst_own=dd_own,
                deg_src_g=dsg,
                deg_src_k=dsk,
            )
        )

    # table-space (G layout) full arrays, same for every core
    deg_src_t = np.ones(TBL, np.float32)
    for c in range(N_CORES):
        deg_src_t[c * NPAD : c * NPAD + NPC] = deg_src[c * NPC : (c + 1) * NPC]
    return cores, deg_src_t, (W, CC, OFF)


def _rsqrt(nc, pool, out, in_ap, tmp_tag):
    """out = 1/sqrt(max(in,1)) elementwise."""
    t1 = pool.tile(list(out.shape), DT, name=f"rs1_{tmp_tag}", tag=f"rs1_{tmp_tag}")
    nc.vector.tensor_scalar_max(t1[:], in_ap, 1.0)
    t2 = pool.tile(list(out.shape), DT, name=f"rs2_{tmp_tag}", tag=f"rs2_{tmp_tag}")
    nc.scalar.sqrt(t2[:], t1[:])
    nc.vector.reciprocal(out, t2[:])


def _build(cfg, dbg=False, timing=False):
    """Build the SPMD Bass program (identical for all cores).

    timing=True replaces AllGathers with local DRAM copies so the module is
    collective-free and TimelineSim-compatible (numerics wrong, timing of
    local work representative)."""
    W, CC, OFF = cfg
    ZROW = 128 * W
    nc = bacc.Bacc(
        "TRN2",
        target_bir_lowering=False,
        debug=False,
        num_devices=1 if timing else N_CORES,
    )
    dbg_specs = {
        "dbg_m0": [128, W],
        "dbg_cs0": [128, W],
        "dbg_agg0": [128, NI],
        "dbg_t0row": [1, NPAD],
        "dbg_t1sl": [128, NI * F2],
        "dbg_T1": [TBL, F2],
        "dbg_m1": [128, W * F2],
        "dbg_agg1": [128, NI * F2],
        "dbg_t2sl": [128, NI],
        "dbg_T2": [TBL, 1],
        "dbg_m2": [128, W],
        "dbg_agg2": [128, NI],
    }
    dbg_t = {}
    if dbg:
        for name, shp in dbg_specs.items():
            dbg_t[name] = nc.dram_tensor(name, shp, DT, kind="ExternalOutput")

    def tap(name, ap):
        if dbg:
            nc.sync.dma_start(dbg_t[name].ap(), ap)
    # ---- I/O -----------------------------------------------------------
    x_t = nc.dram_tensor("x_t", [TBL], DT, kind="ExternalInput")
    deg_src_t = nc.dram_tensor("deg_src_t", [TBL], DT, kind="ExternalInput")
    deg_dst_own = nc.dram_tensor("deg_dst_own", [NPAD], DT, kind="ExternalInput")
    deg_src_g = nc.dram_tensor("deg_src_g", [NPAD], DT, kind="ExternalInput")
    deg_src_k = nc.dram_tensor("deg_src_k", [NPAD], DT, kind="ExternalInput")
    sidx_g = nc.dram_tensor("sidx_g", [128, W], IT, kind="ExternalInput")
    sidx_k = nc.dram_tensor("sidx_k", [128, W], IT, kind="ExternalInput")
    mask_in = nc.dram_tensor("mask", [128, W * 128], DT, kind="ExternalInput")
    bnd_beg = nc.dram_tensor("bnd_beg", [128, NI], IT, kind="ExternalInput")
    bnd_end = nc.dram_tensor("bnd_end", [128, NI], IT, kind="ExternalInput")
    w0 = nc.dram_tensor("w0", [1, F1], DT, kind="ExternalInput")
    b0 = nc.dram_tensor("b0", [F1], DT, kind="ExternalInput")
    w1 = nc.dram_tensor("w1", [F1, F2], DT, kind="ExternalInput")
    b1 = nc.dram_tensor("b1", [F2], DT, kind="ExternalInput")
    w2 = nc.dram_tensor("w2", [F2], DT, kind="ExternalInput")
    b2 = nc.dram_tensor("b2", [1], DT, kind="ExternalInput")
    out = nc.dram_tensor("out", [NPAD], DT, kind="ExternalOutput")

    rg = [list(range(N_CORES))]

    from contextlib import ExitStack

    with tile.TileContext(nc) as tc, ExitStack() as es:
        sb = es.enter_context(tc.tile_pool(name="sb", bufs=1))
        wk = es.enter_context(tc.tile_pool(name="wk", bufs=2))
        pp = es.enter_context(tc.tile_pool(name="pp", bufs=2, space="PSUM"))
        dr = es.enter_context(tc.tile_pool(name="dr", bufs=1, space="DRAM"))

        # ---- persistent DRAM scratch ----------------------------------
        T0_dram = dr.tile([TBL, 1], DT)
        C0_dram = dr.tile([ZROW + 1, 1], DT)
        C1_dram = dr.tile([ZROW + 1, F2], DT)
        C2_dram = dr.tile([ZROW + 1, 1], DT)
        t0_dram = dr.tile([NPAD], DT)
        ag1_in = dr.tile([NPAD, F2], DT)
        T1_dram = dr.tile([TBL, F2], DT, addr_space="Shared")
        ag2_in = dr.tile([NPAD, 1], DT)
        T2_dram = dr.tile([TBL, 1], DT, addr_space="Shared")

        # ---- prep: constants, indices, degree tables ------------------
        ut = sb.tile([P, P], DT)
        make_upper_triangular(nc, ut[:], val=1.0, diag=True)

        idxg = sb.tile([128, W], IT)
        nc.sync.dma_start(idxg[:], sidx_g[:, :])
        idxk = sb.tile([128, W], IT)
        nc.sync.dma_start(idxk[:], sidx_k[:, :])
        bbt = sb.tile([128, NI], IT)
        nc.sync.dma_start(bbt[:], bnd_beg[:, :])
        bet = sb.tile([128, NI], IT)
        nc.sync.dma_start(bet[:], bnd_end[:, :])

        w0sb = sb.tile([1, F1], DT, padded_shape=[128, F1])
        nc.sync.dma_start(w0sb[:], w0[:, :])
        b0col = sb.tile([F1, 1], DT)
        nc.sync.dma_start(b0col[:], b0[:, None])
        w1sb = sb.tile([F1, F2], DT)
        nc.sync.dma_start(w1sb[:], w1[:, :])
        b1rep = sb.tile([P, NI * F2], DT)
        nc.gpsimd.dma_start(
            out=b1rep[:].rearrange("p (a b) -> p a b", b=F2),
            in_=bass.AP(b1.ap().tensor, 0, [[0, P], [0, NI], [1, F2]]),
        )
        w2rep = sb.tile([P, NI * F2], DT)
        nc.gpsimd.dma_start(
            out=w2rep[:].rearrange("p (a b) -> p a b", b=F2),
            in_=bass.AP(w2.ap().tensor, 0, [[0, P], [0, NI], [1, F2]]),
        )
        b2col = sb.tile([P, 1], DT)
        nc.gpsimd.dma_start(
            out=b2col[:], in_=bass.AP(b2.ap().tensor, 0, [[0, P], [1, 1]])
        )

        ddo = sb.tile([P, NI], DT)
        nc.sync.dma_start(ddo[:], deg_dst_own.ap().rearrange("(p i) -> p i", p=128))
        inv_in = sb.tile([P, NI], DT)
        _rsqrt(nc, sb, inv_in[:], ddo[:], "in")
        dsg_t = sb.tile([P, NI], DT)
        nc.sync.dma_start(dsg_t[:], deg_src_g.ap().rearrange("(p i) -> p i", p=128))
        inv_og = sb.tile([P, NI], DT)
        _rsqrt(nc, sb, inv_og[:], dsg_t[:], "og")
        dsk_t = sb.tile([P, NI], DT)
        nc.sync.dma_start(dsk_t[:], deg_src_k.ap().rearrange("(p i) -> p i", p=128))
        inv_ok = sb.tile([P, NI], DT)
        _rsqrt(nc, sb, inv_ok[:], dsk_t[:], "ok")

        # T0 table: x * inv_sqrt(out-degree), all nodes (table space)
        xt_sb = sb.tile([P, TW], DT)
        nc.sync.dma_start(xt_sb[:], x_t.ap().rearrange("(p i) -> p i", p=128))
        dst_sb = sb.tile([P, TW], DT)
        nc.sync.dma_start(dst_sb[:], deg_src_t.ap().rearrange("(p i) -> p i", p=128))
        inv_t = sb.tile([P, TW], DT)
        _rsqrt(nc, sb, inv_t[:], dst_sb[:], "t")
        t0sb = sb.tile([P, TW], DT)
        nc.vector.tensor_tensor(
            out=t0sb[:], in0=xt_sb[:], in1=inv_t[:], op=mybir.AluOpType.mult
        )
        nc.sync.dma_start(
            T0_dram[:, :].rearrange("(p i) f -> p (i f)", p=128), t0sb[:]
        )

        # zero rows of the cumsum scratch buffers
        zrow = sb.tile([1, F2], DT, padded_shape=[128, F2])
        nc.gpsimd.memset(zrow[:], 0.0)
        nc.sync.dma_start(C0_dram[ZROW : ZROW + 1, :], zrow[:, :1])
        nc.sync.dma_start(C1_dram[ZROW : ZROW + 1, :], zrow[:, :])
        nc.sync.dma_start(C2_dram[ZROW : ZROW + 1, :], zrow[:, :1])

        # ---- helpers --------------------------------------------------
        def edge_gather(dst_tile, idx_tile, table, F):
            # HW honors exactly one offset per partition per indirect DMA:
            # one instruction per bin-packed column (128 edges each).
            for s in range(W):
                nc.gpsimd.indirect_dma_start(
                    out=dst_tile[:, s * F : (s + 1) * F],
                    out_offset=None,
                    in_=table[:, :],
                    in_offset=bass.IndirectOffsetOnAxis(
                        ap=idx_tile[:, s : s + 1], axis=0
                    ),
                )

        def cumsum_to_dram(msg_tile, c_dram, F, tag):
            width = W * F
            cs = sb.tile([P, width], DT, name=f"cs_{tag}", tag=f"cs_{tag}")
            step = 510 if F == F2 else 512
            for o in range(0, width, step):
                wn = min(step, width - o)
                ps = pp.tile([P, 512], DT, space="PSUM", tag="cums")
                nc.tensor.matmul(
                    out=ps[:, :wn],
                    lhsT=ut[:],
                    rhs=msg_tile[:, o : o + wn],
                    start=True,
                    stop=True,
                )
                nc.vector.tensor_copy(cs[:, o : o + wn], ps[:, :wn])
            nc.sync.dma_start(
                c_dram[0:ZROW, :].rearrange("(p i) f -> p (i f)", p=128), cs[:]
            )

        def bnd_diff(c_dram, F, tag):
            """gather end/beg rows of c_dram, return (end-beg) tile [P, NI*F]."""
            et = wk.tile([P, NI * F], DT, name=f"e_{tag}", tag=f"e_{tag}")
            bt = wk.tile([P, NI * F], DT, name=f"b_{tag}", tag=f"b_{tag}")
            for s in range(NI):
                nc.gpsimd.indirect_dma_start(
                    out=et[:, s * F : (s + 1) * F],
                    out_offset=None,
                    in_=c_dram[:, :],
                    in_offset=bass.IndirectOffsetOnAxis(ap=bet[:, s : s + 1], axis=0),
                )
                nc.gpsimd.indirect_dma_start(
                    out=bt[:, s * F : (s + 1) * F],
                    out_offset=None,
                    in_=c_dram[:, :],
                    in_offset=bass.IndirectOffsetOnAxis(ap=bbt[:, s : s + 1], axis=0),
                )
            ag = wk.tile([P, NI * F], DT, name=f"ag_{tag}", tag=f"ag_{tag}")
            nc.vector.tensor_tensor(
                out=ag[:], in0=et[:], in1=bt[:], op=mybir.AluOpType.subtract
            )
            return ag

        MCH = 16  # mask-load chunk (columns)

        def agg_mm(msg_tile, F, tag):
            """segment-sum per class via interval-mask matmuls, G-layout."""
            ps = pp.tile([P, 512], DT, space="PSUM", tag="aggps")
            for c0 in range(0, W, MCH):
                cw = min(MCH, W - c0)
                mkt = wk.tile([P, MCH * 128], DT, name=f"mk_{tag}", tag="mk")
                nc.scalar.dma_start(
                    mkt[:, :cw * 128], mask_in[:, c0 * 128:(c0 + cw) * 128]
                )
                for c in range(c0, c0 + cw):
                    r = int(np.searchsorted(OFF, c, side="right")) - 1
                    k = c - int(OFF[r])
                    nc.tensor.matmul(
                        out=ps[:, r * F:(r + 1) * F],
                        lhsT=mkt[:, (c - c0) * 128:(c - c0 + 1) * 128],
                        rhs=msg_tile[:, c * F:(c + 1) * F],
                        start=(k == 0),
                        stop=(k == int(CC[r]) - 1),
                        skip_group_check=True,
                    )
            ag = wk.tile([P, NI * F], DT, name=f"ag_{tag}", tag=f"ag_{tag}")
            nc.vector.tensor_copy(ag[:], ps[:, :NI * F])
            return ag

        # ---- layer 0 (F=1) -------------------------------------------
        m0 = sb.tile([P, W], DT)
        edge_gather(m0, idxg, T0_dram, 1)
        tap("dbg_m0", m0[:])
        agg0 = agg_mm(m0, 1, "l0")
        tap("dbg_agg0", agg0[:])
        t0n = wk.tile([P, NI], DT)
        nc.vector.tensor_tensor(
            out=t0n[:], in0=agg0[:], in1=inv_in[:], op=mybir.AluOpType.mult
        )
        nc.sync.dma_start(t0_dram[:].rearrange("(p i) -> p i", p=128), t0n[:])
        t0row = sb.tile([1, NPAD], DT)
        nc.sync.dma_start(t0row[:], t0_dram[:])
        tap("dbg_t0row", t0row[:])

        # dense chain: h1 = lrelu(t0 x W0 + b0); msg1 = inv_out*(h1 @ W1)
        t1sl = sb.tile([P, NI * F2], DT)
        for o in range(0, NPAD, 512):
            wn = min(512, NPAD - o)
            ps1 = pp.tile([F1, 512], DT, space="PSUM", tag="ps1")
            nc.tensor.matmul(
                out=ps1[:, :wn],
                lhsT=w0sb[:],
                rhs=t0row[:, o : o + wn],
                start=True,
                stop=True,
            )
            xb = wk.tile([F1, 512], DT, tag="xb")
            nc.scalar.activation(
                xb[:, :wn],
                ps1[:, :wn],
                mybir.ActivationFunctionType.Identity,
                bias=b0col[:],
            )
            x01 = wk.tile([F1, 512], DT, tag="x01")
            nc.vector.tensor_scalar(
                out=x01[:, :wn],
                in0=ps1[:, :wn],
                scalar1=b0col[:],
                scalar2=0.01,
                op0=mybir.AluOpType.add,
                op1=mybir.AluOpType.mult,
            )
            h1c = wk.tile([F1, 512], DT, tag="h1c")
            nc.vector.tensor_tensor(
                out=h1c[:, :wn], in0=xb[:, :wn], in1=x01[:, :wn],
                op=mybir.AluOpType.max,
            )
            for sub in range(0, wn, 128):
                k = (o + sub) // 128
                ps3 = pp.tile([P, F2], DT, space="PSUM", tag="ps3")
                nc.tensor.matmul(
                    out=ps3[:],
                    lhsT=h1c[:, sub : sub + 128],
                    rhs=w1sb[:],
                    start=True,
                    stop=True,
                )
                nc.vector.tensor_scalar_mul(
                    t1sl[:, k * F2 : (k + 1) * F2], ps3[:], inv_ok[:, k : k + 1]
                )
        tap("dbg_t1sl", t1sl[:])
        nc.sync.dma_start(
            ag1_in[:, :].rearrange("(p i) f -> p (i f)", p=128), t1sl[:]
        )
        if timing:
            nc.sync.dma_start(T1_dram[0:NPAD, :], ag1_in[:, :])
        else:
            nc.gpsimd.collective_compute(
                "AllGather",
                mybir.AluOpType.bypass,
                replica_groups=rg,
                ins=[ag1_in.opt()],
                outs=[T1_dram.opt()],
            )
        tap("dbg_T1", T1_dram[:, :])

        # ---- layer 1 (F=10) ------------------------------------------
        m1 = sb.tile([P, W * F2], DT)
        edge_gather(m1, idxk, T1_dram, F2)
        tap("dbg_m1", m1[:])
        agg1 = agg_mm(m1, F2, "l1")
        tap("dbg_agg1", agg1[:])
        inv_in_rep = bass.AP(
            inv_in[:].tensor, inv_in[:].offset,
            [inv_in[:].ap[0], [1, NI], [0, F2]],
        )
        mm1 = wk.tile([P, NI * F2], DT)
        nc.vector.tensor_tensor(
            out=mm1[:].rearrange("p (a b) -> p a b", b=F2),
            in0=agg1[:].rearrange("p (a b) -> p a b", b=F2),
            in1=inv_in_rep,
            op=mybir.AluOpType.mult,
        )
        h2a = wk.tile([P, NI * F2], DT)
        nc.vector.tensor_tensor(
            out=h2a[:], in0=mm1[:], in1=b1rep[:], op=mybir.AluOpType.add
        )
        h2 = wk.tile([P, NI * F2], DT)
        nc.vector.tensor_scalar_max(h2[:], h2a[:], 0.0)
        # msg2 = inv_out_g * (h2 @ W2)
        hw2 = wk.tile([P, NI * F2], DT)
        nc.vector.tensor_tensor(
            out=hw2[:], in0=h2[:], in1=w2rep[:], op=mybir.AluOpType.mult
        )
        red = wk.tile([P, NI], DT)
        nc.vector.reduce_sum(
            red[:, :, None],
            hw2[:].rearrange("p (a b) -> p a b", b=F2),
            axis=mybir.AxisListType.X,
        )
        t2sl = sb.tile([P, NI], DT)
        nc.vector.tensor_tensor(
            out=t2sl[:], in0=red[:], in1=inv_og[:], op=mybir.AluOpType.mult
        )
        tap("dbg_t2sl", t2sl[:])
        nc.sync.dma_start(
            ag2_in[:, :].rearrange("(p i) f -> p (i f)", p=128), t2sl[:]
        )
        if timing:
            nc.sync.dma_start(T2_dram[0:NPAD, :], ag2_in[:, :])
        else:
            nc.gpsimd.collective_compute(
                "AllGather",
                mybir.AluOpType.bypass,
                replica_groups=rg,
                ins=[ag2_in.opt()],
                outs=[T2_dram.opt()],
            )
        tap("dbg_T2", T2_dram[:, :])

        # ---- layer 2 (F=1) -------------------------------------------
        m2 = sb.tile([P, W], DT)
        edge_gather(m2, idxg, T2_dram, 1)
        tap("dbg_m2", m2[:])
        agg2 = agg_mm(m2, 1, "l2")
        tap("dbg_agg2", agg2[:])
        t2n = wk.tile([P, NI], DT)
        nc.vector.tensor_tensor(
            out=t2n[:], in0=agg2[:], in1=inv_in[:], op=mybir.AluOpType.mult
        )
        h3 = wk.tile([P, NI], DT)
        nc.vector.tensor_scalar(
            out=h3[:],
            in0=t2n[:],
            scalar1=b2col[:],
            scalar2=0.0,
            op0=mybir.AluOpType.add,
            op1=mybir.AluOpType.max,
        )
        nc.sync.dma_start(out.ap().rearrange("(p i) -> p i", p=128), h3[:])

    nc.compile()
    return nc


def build_in_maps(in_feat, edge_index, W0, b0, W1, b1, W2, b2):
    cores, deg_src_t, cfg = _preprocess(np.asarray(edge_index))
    x = np.asarray(in_feat, np.float32).reshape(-1)
    x_t = np.zeros(TBL, np.float32)
    for c in range(N_CORES):
        x_t[c * NPAD : c * NPAD + NPC] = x[c * NPC : (c + 1) * NPC]
    common = dict(
        x_t=x_t,
        deg_src_t=deg_src_t,
        w0=np.asarray(W0, np.float32).reshape(1, F1),
        b0=np.asarray(b0, np.float32).reshape(F1),
        w1=np.asarray(W1, np.float32).reshape(F1, F2),
        b1=np.asarray(b1, np.float32).reshape(F2),
        w2=np.asarray(W2, np.float32).reshape(F2),
        b2=np.asarray(b2, np.float32).reshape(1),
    )
    in_maps = []
    for c in range(N_CORES):
        d = cores[c]
        in_maps.append(
            dict(
                common,
                deg_dst_own=d["deg_dst_own"],
                deg_src_g=d["deg_src_g"],
                deg_src_k=d["deg_src_k"],
                sidx_g=d["sidx_g"],
                sidx_k=d["sidx_k"],
                mask=d["mask"],
                bnd_beg=d["bnd_beg"],
                bnd_end=d["bnd_end"],
            )
        )
    return in_maps, cfg


def assemble(results):
    full = np.zeros((N_NODES, 1), np.float32)
    for c in range(N_CORES):
        full[c * NPC : (c + 1) * NPC, 0] = results[c]["out"][:NPC]
    return full


def kernel(in_feat, edge_index, W0, b0, W1, b1, W2, b2):
    in_maps, cfg = build_in_maps(in_feat, edge_index, W0, b0, W1, b1, W2, b2)
    nc = _build(cfg)
    res = run_bass_kernel_spmd(
        nc, in_maps, core_ids=list(range(N_CORES)), trace=False
    )
    return assemble(res.results)



# revision 23
# speedup vs baseline: 7.4143x; 7.4143x over previous
"""GCN (3-layer DGL GraphConv, norm='both') on 8 TRN2 NeuronCores.

Push-mode design: each core owns a contiguous range of 6250 src nodes.
Per layer:
  - per-edge messages are built ON-CHIP: node values are placed at their
    edge-run starts with gpsimd local_scatter (per-partition static
    indices), then expanded across each run by a masked prefix scan on DVE
    (state = mask*state + scattered_val) -- zero DMA descriptors.
  - segment-sum over dst runs as SBUF-destination dma_scatter_add
    (parity-split CCE f32 accumulate): ~21k edge tokens per instruction
    instead of 128 per indirect DMA, amortizing the 994ns SWDGE fixed
    overhead ~170x.
  - a ReduceScatter(add) over the 8 cores hands every core its own
    6272-slot aggregation slice; no AllGather / replicated tables needed.
Host does index-only preprocessing (sort, run starts, token->dst index
tables); all float math runs on device.

Token layout per core: lane p = (src % 6250) % 128, so a node's value and
all its edge tokens live on the same partition. Tokens are grouped into an
A region (dst slot < 32768) and a B region (rest) so scatter indices fit
int16; ghost tokens are forced to 0.0 by the mask and scatter into slot 0
(+= 0.0, harmless).
"""

import sys

import numpy as np

try:
    import concourse.bass as bass  # noqa: F401
except Exception:  # pragma: no cover
    sys.path.insert(0, "/opt/trn_rl_repo")

import concourse.bass as bass
import concourse.bacc as bacc
import concourse.tile as tile
from concourse import mybir
from concourse.bass_utils import run_bass_kernel_spmd

N_CORES = 8
N_NODES = 50000
NPC = N_NODES // N_CORES   # 6250 real nodes per core
KC = 49                    # node columns per lane
NPAD = KC * 128            # 6272 padded slots per core
TBL = N_CORES * NPAD       # 50176 global slots
ASPLIT = 32768             # dst slots < ASPLIT scatter in region A
BSIZE = TBL - ASPLIT       # 17408
F1 = 100
F2 = 10
P = 128
CHCOLS = 84                # scatter chunk: 84*128 = 10752 tokens
DT = mybir.dt.float32
HT = mybir.dt.float16
IT = mybir.dt.int16


def _preprocess(edge_index):
    """Index-only host prep. Returns per-core static arrays + (cA, cB)."""
    src = edge_index[0].astype(np.int64)
    dst = edge_index[1].astype(np.int64)
    deg_out = np.bincount(src, minlength=N_NODES).astype(np.float32)
    deg_in = np.bincount(dst, minlength=N_NODES).astype(np.float32)

    own = src // NPC
    l = src % NPC
    lane = l % 128
    dslot = (dst // NPC) * NPAD + dst % NPC
    region = (dslot >= ASPLIT).astype(np.int64)

    # ---- pass 1: global cA / cB (shapes shared by the SPMD program) ----
    key_lane = own * 128 + lane
    nA = np.bincount(key_lane[region == 0], minlength=N_CORES * 128)
    nB = np.bincount(key_lane[region == 1], minlength=N_CORES * 128)
    cA = int(nA.max())
    cB = int(nB.max())
    cA += (-cA) % 2
    cB += (-cB) % 2
    C = cA + cB

    cores = []
    for c in range(N_CORES):
        m = own == c
        l_c, lane_c, reg_c, ds_c = l[m], lane[m], region[m], dslot[m]
        order = np.lexsort((ds_c, l_c, reg_c, lane_c))
        l_s, lane_s, reg_s, ds_s = (
            l_c[order], lane_c[order], reg_c[order], ds_c[order],
        )
        # position of each token within its (lane, region) group
        grp = lane_s * 2 + reg_s
        E_c = len(grp)
        cnt = np.bincount(grp, minlength=256)
        gstart = np.zeros(257, np.int64)
        np.cumsum(cnt, out=gstart[1:])
        pos = np.arange(E_c) - gstart[grp]
        # run starts: first token of each (lane, region, node)
        newrun = np.ones(E_c, bool)
        if E_c > 1:
            same = (
                (lane_s[1:] == lane_s[:-1])
                & (reg_s[1:] == reg_s[:-1])
                & (l_s[1:] == l_s[:-1])
            )
            newrun[1:] = ~same
        # column within the full [0, C) token space
        col = pos + np.where(reg_s == 1, cA, 0)

        mask = np.ones((128, C), np.float32)
        mask[lane_s[newrun], col[newrun]] = 0.0
        nA_l = cnt[0::2]
        nB_l = cnt[1::2]
        for pp in range(128):
            mask[pp, nA_l[pp]:cA] = 0.0
            mask[pp, cA + nB_l[pp]:] = 0.0

        ls_idx = np.full((128, 2 * KC), -1, np.int16)
        k_s = l_s // 128
        ls_idx[lane_s[newrun], (k_s[newrun] + KC * reg_s[newrun])] = col[
            newrun
        ].astype(np.int16)

        def tok_table(ncols, reg, base):
            idx = np.zeros((128, ncols), np.int16)  # ghosts -> slot 0
            sel = reg_s == reg
            idx[lane_s[sel], pos[sel]] = (ds_s[sel] - base).astype(np.int16)
            # wrap: token i=(p=i%128, col=i//128) -> [i%16, i//16]
            flat = idx.T.reshape(-1)  # token order: col-major over (col, p)
            n_tok = 128 * ncols
            wrapped = np.zeros((16, n_tok // 16), np.int16)
            wrapped[np.arange(n_tok) % 16, np.arange(n_tok) // 16] = flat
            return np.tile(wrapped, (8, 1))

        tikA = tok_table(cA, 0, 0)
        tikB = tok_table(cB, 1, ASPLIT)

        dg_o = np.zeros((128, KC), np.float32)
        dg_i = np.ones((128, KC), np.float32)
        ll = np.arange(NPC)
        dg_o[ll % 128, ll // 128] = deg_out[c * NPC:(c + 1) * NPC]
        dg_i[ll % 128, ll // 128] = deg_in[c * NPC:(c + 1) * NPC]
        cores.append(
            dict(mask=mask, ls_idx=ls_idx, tikA=tikA, tikB=tikB,
                 deg_out=dg_o, deg_in=dg_i)
        )
    return cores, (cA, cB)


def _rsqrt(nc, pool, out, in_ap, tag):
    """out = 1/sqrt(max(in,1)) elementwise."""
    t1 = pool.tile(list(out.shape), DT, name=f"rs1_{tag}", tag=f"rs1_{tag}")
    nc.vector.tensor_scalar_max(t1[:], in_ap, 1.0)
    t2 = pool.tile(list(out.shape), DT, name=f"rs2_{tag}", tag=f"rs2_{tag}")
    nc.scalar.sqrt(t2[:], t1[:])
    nc.vector.reciprocal(out, t2[:])


def _build(cfg, timing=False):
    """Build the SPMD Bass program (identical for all cores).

    timing=True replaces the ReduceScatters with local DRAM copies so the
    module is collective-free and TimelineSim-compatible."""
    cA, cB = cfg
    C = cA + cB
    nc = bacc.Bacc(
        "TRN2",
        target_bir_lowering=False,
        debug=False,
        num_devices=1 if timing else N_CORES,
        dynamic_dma_scratch_size=2**16,
    )

    # ---- I/O -----------------------------------------------------------
    x_own = nc.dram_tensor("x_own", [P, KC], DT, kind="ExternalInput")
    deg_out = nc.dram_tensor("deg_out", [P, KC], DT, kind="ExternalInput")
    deg_in = nc.dram_tensor("deg_in", [P, KC], DT, kind="ExternalInput")
    mask_d = nc.dram_tensor("mask", [P, C], DT, kind="ExternalInput")
    lsx_d = nc.dram_tensor("ls_idx", [P, 2 * KC], IT, kind="ExternalInput")
    tikA_d = nc.dram_tensor("tikA", [P, 8 * cA], IT, kind="ExternalInput")
    tikB_d = nc.dram_tensor("tikB", [P, 8 * cB], IT, kind="ExternalInput")
    w0 = nc.dram_tensor("w0", [1, F1], DT, kind="ExternalInput")
    b0 = nc.dram_tensor("b0", [F1], DT, kind="ExternalInput")
    w1 = nc.dram_tensor("w1", [F1, F2], DT, kind="ExternalInput")
    b1 = nc.dram_tensor("b1", [F2], DT, kind="ExternalInput")
    w2 = nc.dram_tensor("w2", [F2], DT, kind="ExternalInput")
    b2 = nc.dram_tensor("b2", [1], DT, kind="ExternalInput")
    out = nc.dram_tensor("out", [NPAD], DT, kind="ExternalOutput")

    rg = [list(range(N_CORES))]
    NGA = 128  # accA groups (slots 0..255 in parity pairs)
    NGB = 68   # accB groups

    from contextlib import ExitStack

    with tile.TileContext(nc) as tc, ExitStack() as es:
        sb = es.enter_context(tc.tile_pool(name="sb", bufs=1))
        wk = es.enter_context(tc.tile_pool(name="wk", bufs=2))
        pp = es.enter_context(tc.tile_pool(name="pp", bufs=2, space="PSUM"))
        dr = es.enter_context(tc.tile_pool(name="dr", bufs=1, space="DRAM"))

        cc0 = dr.tile([TBL], DT)
        cc1 = dr.tile([TBL * F2], DT)
        cc2 = dr.tile([TBL], DT)
        rs0 = dr.tile([NPAD], DT)
        rs1 = dr.tile([NPAD * F2], DT)
        rs2 = dr.tile([NPAD], DT)
        t0_dram = dr.tile([NPAD], DT)

        # ---- static tables / weights ----------------------------------
        mk = sb.tile([P, C], DT)
        nc.sync.dma_start(mk[:], mask_d[:, :])
        lsx = sb.tile([P, 2 * KC], IT)
        nc.sync.dma_start(lsx[:], lsx_d[:, :])
        tikA = sb.tile([P, 8 * cA], IT)
        nc.sync.dma_start(tikA[:], tikA_d[:, :])
        tikB = sb.tile([P, 8 * cB], IT)
        nc.sync.dma_start(tikB[:], tikB_d[:, :])

        w0sb = sb.tile([1, F1], DT, padded_shape=[128, F1])
        nc.sync.dma_start(w0sb[:], w0[:, :])
        b0col = sb.tile([F1, 1], DT)
        nc.sync.dma_start(b0col[:], b0[:, None])
        w1sb = sb.tile([F1, F2], DT)
        nc.sync.dma_start(w1sb[:], w1[:, :])
        b1rep = sb.tile([P, KC * F2], DT)
        nc.gpsimd.dma_start(
            out=b1rep[:].rearrange("p (a b) -> p a b", b=F2),
            in_=bass.AP(b1.ap().tensor, 0, [[0, P], [0, KC], [1, F2]]),
        )
        w2rep = sb.tile([P, KC * F2], DT)
        nc.gpsimd.dma_start(
            out=w2rep[:].rearrange("p (a b) -> p a b", b=F2),
            in_=bass.AP(w2.ap().tensor, 0, [[0, P], [0, KC], [1, F2]]),
        )
        b2col = sb.tile([P, 1], DT)
        nc.gpsimd.dma_start(
            out=b2col[:], in_=bass.AP(b2.ap().tensor, 0, [[0, P], [1, 1]])
        )

        dgo = sb.tile([P, KC], DT)
        nc.sync.dma_start(dgo[:], deg_out[:, :])
        inv_out = sb.tile([P, KC], DT)
        _rsqrt(nc, sb, inv_out[:], dgo[:], "o")
        dgi = sb.tile([P, KC], DT)
        nc.sync.dma_start(dgi[:], deg_in[:, :])
        inv_in = sb.tile([P, KC], DT)
        _rsqrt(nc, sb, inv_in[:], dgi[:], "i")

        xo = sb.tile([P, KC], DT)
        nc.sync.dma_start(xo[:], x_own[:, :])

        # ---- shared scratch for expansion/scatter ---------------------
        # Two accumulator sets ping-ponged across scatter chunks: same-set
        # chunks are serialized on their DMA sem (WAW-safe), adjacent
        # chunks overlap desc-gen with DMA transfer.
        msg = sb.tile([P, C * F2], DT)          # reused by all layers
        accA = [
            [sb.tile([P, NGA * F2], DT, name=f"accA{w}{s}") for w in "op"]
            for s in (0, 1)
        ]
        accB = [
            [sb.tile([P, NGB * F2], DT, name=f"accB{w}{s}") for w in "op"]
            for s in (0, 1)
        ]

        def expand(t_lane, F, tag):
            """t_lane [P, KC*F] (k-major, f inner) -> msg[:, :C*F] tokens."""
            t16 = wk.tile([P, KC * F], HT, name=f"t16_{tag}", tag="t16")
            nc.vector.tensor_copy(t16[:], t_lane)
            for f in range(F):
                src_f = t16[:, f::F] if F > 1 else t16[:, :]
                d98 = wk.tile([P, 2 * KC], HT, name=f"d98_{tag}{f}", tag="d98")
                nc.scalar.copy(d98[:, 0:KC], src_f)
                nc.scalar.copy(d98[:, KC:2 * KC], src_f)
                sc = wk.tile([P, C], HT, name=f"sc_{tag}{f}", tag="scat")
                nc.gpsimd.local_scatter(
                    out_ap=sc[:], data_ap=d98[:], idxs_ap=lsx[:],
                    channels=P, num_elems=C, num_idxs=2 * KC,
                )
                out_ap = msg[:, f:C * F:F] if F > 1 else msg[:, 0:C]
                nc.vector.tensor_tensor_scan(
                    out=out_ap, data0=mk[:], data1=sc[:], initial=0.0,
                    op0=mybir.AluOpType.mult, op1=mybir.AluOpType.add,
                )

        def scatter_layer(F):
            """Scatter msg tokens into the ping-pong accumulator sets.

            Chunks alternate sets, so the tile framework's DMASW-lane
            dependency tracking orders same-set (same-tile) writes while
            adjacent chunks overlap desc-gen with DMA transfer. All
            synchronization (WAW between chunks, RAW to the combine) is
            tile-framework-managed."""
            for s in (0, 1):
                for t in accA[s]:
                    nc.vector.memset(t[:, : NGA * F], 0.0)
                for t in accB[s]:
                    nc.vector.memset(t[:, : NGB * F], 0.0)
            ci = 0
            for reg, ncols, tik, ngrp in (
                (0, cA, tikA, NGA),
                (1, cB, tikB, NGB),
            ):
                base = 0 if reg == 0 else cA
                accs = accA if reg == 0 else accB
                for c0 in range(0, ncols, CHCOLS):
                    cw = min(CHCOLS, ncols - c0)
                    s = ci % 2
                    ci += 1
                    in_ap = msg[
                        :, (base + c0) * F: (base + c0 + cw) * F
                    ].rearrange("p (c f) -> p c f", f=F)
                    nc.gpsimd.dma_scatter_add(
                        out_ap=accs[s][0][:, : ngrp * F].rearrange(
                            "p (g f) -> p g f", f=F),
                        in_ap=in_ap,
                        idxs_ap=tik[:, 8 * c0: 8 * (c0 + cw)],
                        num_idxs=128 * cw,
                        num_idxs_reg=128 * cw,
                        elem_size=F,
                        sbuf_tokens_per_rank=128,
                        parity_reg=0,
                        out_ap_other=accs[s][1][:, : ngrp * F].rearrange(
                            "p (g f) -> p g f", f=F),
                    )

        def reduce_scatter(F, cc, rs, tag):
            """accs -> cc (canonical [TBL, F]) -> ReduceScatter -> rs."""
            # combine set 1 into set 0 (tc syncs on the scatter DMAs)
            for pair in (accA, accB):
                ngrp = NGA if pair is accA else NGB
                for w in (0, 1):
                    nc.vector.tensor_tensor(
                        out=pair[0][w][:, : ngrp * F],
                        in0=pair[0][w][:, : ngrp * F],
                        in1=pair[1][w][:, : ngrp * F],
                        op=mybir.AluOpType.add,
                    )
            for par, acc, ngrp, off in (
                (0, accA[0][0], NGA, 0),
                (1, accA[0][1], NGA, 0),
                (0, accB[0][0], NGB, ASPLIT),
                (1, accB[0][1], NGB, ASPLIT),
            ):
                dst = bass.AP(
                    cc[:].tensor,
                    cc[:].offset + (off + par * 128) * F,
                    [[F, 128], [256 * F, ngrp], [1, F]],
                )
                nc.sync.dma_start(
                    dst, acc[:, : ngrp * F].rearrange("p (g f) -> p g f", f=F)
                )
            if timing:
                nc.sync.dma_start(rs[:], cc[0: NPAD * F])
            else:
                nc.gpsimd.collective_compute(
                    "ReduceScatter",
                    mybir.AluOpType.add,
                    replica_groups=rg,
                    ins=[cc.opt()],
                    outs=[rs.opt()],
                )
            # rs [NPAD*F] -> lane tile [P, KC*F]: (l,f) at l=k*128+p
            lane = sb.tile([P, KC * F], DT, name=f"agg_{tag}", tag=f"agg_{tag}")
            src = bass.AP(
                rs[:].tensor, rs[:].offset,
                [[F, 128], [128 * F, KC], [1, F]],
            )
            nc.sync.dma_start(lane[:].rearrange("p (k f) -> p k f", f=F), src)
            return lane


        # ================= layer 0 (F=1) ===============================
        t0 = wk.tile([P, KC], DT, tag="t0")
        nc.vector.tensor_tensor(
            out=t0[:], in0=xo[:], in1=inv_out[:], op=mybir.AluOpType.mult
        )
        expand(t0[:], 1, "l0")
        scatter_layer(1)
        agg0 = reduce_scatter(1, cc0, rs0, "l0")

        # t0n = agg0 * inv_in -> flat DRAM -> dense chain -> t1sl
        t0n = wk.tile([P, KC], DT, tag="t0n")
        nc.vector.tensor_tensor(
            out=t0n[:], in0=agg0[:], in1=inv_in[:], op=mybir.AluOpType.mult
        )
        nc.sync.dma_start(
            t0_dram[:].rearrange("(k p) -> p k", p=128), t0n[:]
        )
        t0row = sb.tile([1, NPAD], DT)
        nc.sync.dma_start(t0row[:], t0_dram[:])

        t1sl = sb.tile([P, KC * F2], DT)
        for o in range(0, NPAD, 512):
            wn = min(512, NPAD - o)
            ps1 = pp.tile([F1, 512], DT, space="PSUM", tag="ps1")
            nc.tensor.matmul(
                out=ps1[:, :wn], lhsT=w0sb[:], rhs=t0row[:, o: o + wn],
                start=True, stop=True,
            )
            xb = wk.tile([F1, 512], DT, tag="xb")
            nc.scalar.activation(
                xb[:, :wn], ps1[:, :wn],
                mybir.ActivationFunctionType.Identity, bias=b0col[:],
            )
            x01 = wk.tile([F1, 512], DT, tag="x01")
            nc.vector.tensor_scalar(
                out=x01[:, :wn], in0=ps1[:, :wn],
                scalar1=b0col[:], scalar2=0.01,
                op0=mybir.AluOpType.add, op1=mybir.AluOpType.mult,
            )
            h1c = wk.tile([F1, 512], DT, tag="h1c")
            nc.vector.tensor_tensor(
                out=h1c[:, :wn], in0=xb[:, :wn], in1=x01[:, :wn],
                op=mybir.AluOpType.max,
            )
            for sub in range(0, wn, 128):
                k = (o + sub) // 128
                ps3 = pp.tile([P, F2], DT, space="PSUM", tag="ps3")
                nc.tensor.matmul(
                    out=ps3[:], lhsT=h1c[:, sub: sub + 128], rhs=w1sb[:],
                    start=True, stop=True,
                )
                nc.vector.tensor_scalar_mul(
                    t1sl[:, k * F2: (k + 1) * F2], ps3[:],
                    inv_out[:, k: k + 1],
                )

        # ================= layer 1 (F=10) ==============================
        expand(t1sl[:], F2, "l1")
        scatter_layer(F2)
        agg1 = reduce_scatter(F2, cc1, rs1, "l1")

        inv_in_rep = bass.AP(
            inv_in[:].tensor, inv_in[:].offset,
            [inv_in[:].ap[0], [1, KC], [0, F2]],
        )
        mm1 = wk.tile([P, KC * F2], DT, tag="mm1")
        nc.vector.tensor_tensor(
            out=mm1[:].rearrange("p (a b) -> p a b", b=F2),
            in0=agg1[:].rearrange("p (a b) -> p a b", b=F2),
            in1=inv_in_rep,
            op=mybir.AluOpType.mult,
        )
        h2a = wk.tile([P, KC * F2], DT, tag="h2a")
        nc.vector.tensor_tensor(
            out=h2a[:], in0=mm1[:], in1=b1rep[:], op=mybir.AluOpType.add
        )
        h2 = wk.tile([P, KC * F2], DT, tag="h2")
        nc.vector.tensor_scalar_max(h2[:], h2a[:], 0.0)
        hw2 = wk.tile([P, KC * F2], DT, tag="hw2")
        nc.vector.tensor_tensor(
            out=hw2[:], in0=h2[:], in1=w2rep[:], op=mybir.AluOpType.mult
        )
        red = wk.tile([P, KC], DT, tag="red")
        nc.vector.reduce_sum(
            red[:, :, None],
            hw2[:].rearrange("p (a b) -> p a b", b=F2),
            axis=mybir.AxisListType.X,
        )
        t2 = wk.tile([P, KC], DT, tag="t2")
        nc.vector.tensor_tensor(
            out=t2[:], in0=red[:], in1=inv_out[:], op=mybir.AluOpType.mult
        )

        # ================= layer 2 (F=1) ===============================
        expand(t2[:], 1, "l2")
        scatter_layer(1)
        agg2 = reduce_scatter(1, cc2, rs2, "l2")

        t2n = wk.tile([P, KC], DT, tag="t2n")
        nc.vector.tensor_tensor(
            out=t2n[:], in0=agg2[:], in1=inv_in[:], op=mybir.AluOpType.mult
        )
        h3 = wk.tile([P, KC], DT, tag="h3")
        nc.vector.tensor_scalar(
            out=h3[:], in0=t2n[:],
            scalar1=b2col[:], scalar2=0.0,
            op0=mybir.AluOpType.add, op1=mybir.AluOpType.max,
        )
        nc.sync.dma_start(out.ap().rearrange("(k p) -> p k", p=128), h3[:])

    nc.compile()
    return nc


def build_in_maps(in_feat, edge_index, W0, b0, W1, b1, W2, b2):
    cores, cfg = _preprocess(np.asarray(edge_index))
    x = np.asarray(in_feat, np.float32).reshape(-1)
    common = dict(
        w0=np.asarray(W0, np.float32).reshape(1, F1),
        b0=np.asarray(b0, np.float32).reshape(F1),
        w1=np.asarray(W1, np.float32).reshape(F1, F2),
        b1=np.asarray(b1, np.float32).reshape(F2),
        w2=np.asarray(W2, np.float32).reshape(F2),
        b2=np.asarray(b2, np.float32).reshape(1),
    )
    in_maps = []
    ll = np.arange(NPC)
    for c in range(N_CORES):
        d = cores[c]
        xo = np.zeros((128, KC), np.float32)
        xo[ll % 128, ll // 128] = x[c * NPC:(c + 1) * NPC]
        in_maps.append(
            dict(
                common,
                x_own=xo,
                deg_out=d["deg_out"],
                deg_in=d["deg_in"],
                mask=d["mask"],
                ls_idx=d["ls_idx"],
                tikA=d["tikA"],
                tikB=d["tikB"],
            )
        )
    return in_maps, cfg


def assemble(results):
    full = np.zeros((N_NODES, 1), np.float32)
    ll = np.arange(NPC)
    for c in range(N_CORES):
        o = results[c]["out"]
        full[c * NPC:(c + 1) * NPC, 0] = o.reshape(-1)[:NPC]
    return full


def kernel(in_feat, edge_index, W0, b0, W1, b1, W2, b2):
    in_maps, cfg = build_in_maps(in_feat, edge_index, W0, b0, W1, b1, W2, b2)
    nc = _build(cfg)
    res = run_bass_kernel_spmd(
        nc, in_maps, core_ids=list(range(N_CORES)), trace=False
    )
    return assemble(res.results)
